# revision 1
# baseline (speedup 1.0000x reference)
"""Trainium2 Bass kernel for the pair-span GNN message-passing model, v2.

Math (per batch element b):
    W1..W4 = split(Wcat); A' = h @ (W1+W3) + bW;  Bm = h @ (W2-W3)
    For each triu pair p=(i,j):  spans[p] = tanh(A'[i] + Bm[j] + (h_i*h_j) @ W4)
    alpha = softmax(spans @ h_hat);  h_tilde = alpha^T spans
    out = log_softmax(h_tilde @ Wout + bout)

Sharding: data-parallel over batch B=8 across the 8 NeuronCores.

v2 minimizes host->device traffic (the dominant cost in this setup):
  - A' and Bm are precomputed on host (cheap BLAS) so W13/W23 never ship.
  - The 0/1 pair-selection matrices are built on device per tile (row
    memsets for the i-run selector, identity-column copies for the
    shifted-diagonal j selector) instead of shipping 4.5MB per core.
  - Single merged output tensor [128, HC+1] (acc columns + denominator).
  - No gpsimd: the softmax rescale broadcast uses a ones-outer matmul.
Per-core inputs: hTn 196KB + A16/Bm16 392KB + W4 1.18MB + ident 32KB.
"""

import numpy as np

import concourse.bacc as bacc
import concourse.bass as bass
import concourse.mybir as mybir
from concourse import tile as tile_mod
from concourse.bass_utils import run_bass_kernel_spmd

B, N, H, C = 8, 128, 768, 5
HC = H // 128          # 6 chunks of the hidden dim
P_TOT = N * (N + 1) // 2   # 8256 pairs
PT = 512               # pairs per tile
NT = (P_TOT + PT - 1) // PT  # 17 tiles (last has 64 pairs)

F16 = mybir.dt.float16
F32 = mybir.dt.float32
NPMM = np.float16      # host dtype for matmul operands


def _tile_width(t: int) -> int:
    return min(PT, P_TOT - t * PT)


def _segments():
    """(tile, i, j0, p0_local, length) runs of pairs; block i = pairs (i, j>=i)."""
    segs = []
    p = 0
    for i in range(N):
        j0 = i
        rem = N - i
        while rem > 0:
            t, p0 = divmod(p, PT)
            take = min(rem, PT - p0)
            segs.append((t, i, j0, p0, take))
            p += take
            j0 += take
            rem -= take
    return segs


_SEGS = _segments()


def build_nc(nt: int = NT) -> bass.Bass:
    # Bacc (not plain Bass): its finalize() runs generate_event_semaphores,
    # which splits multi-sem waits into standalone InstEventSemaphore ops —
    # the DVE ISA structs only fit one wait each.
    nc = bacc.Bacc(None)
    AF = mybir.ActivationFunctionType
    OP = mybir.AluOpType

    hTn_d = nc.declare_dram_parameter("hTn", [128, N, HC], F16, isOutput=False)
    W4_d = nc.declare_dram_parameter("W4p", [128, HC, H], F16, isOutput=False)
    A16_d = nc.declare_dram_parameter("A16", [128, H], F16, isOutput=False)
    Bm16_d = nc.declare_dram_parameter("Bm16", [128, H], F16, isOutput=False)
    hhat_d = nc.declare_dram_parameter("hhatT", [128, HC], F16, isOutput=False)
    id_d = nc.declare_dram_parameter("ident", [128, 128], F16, isOutput=False)
    out_d = nc.declare_dram_parameter("out", [128, HC + 1], F32, isOutput=True)

    with tile_mod.TileContext(nc) as tc:
        with (
            tc.tile_pool(name="const", bufs=1) as cpool,
            tc.tile_pool(name="work", bufs=2) as wpool,
            tc.tile_pool(name="mpsum", bufs=1, space="PSUM") as mpsum,
        ):
            # ---- static loads (all fully contiguous) ----
            hTn = cpool.tile([128, N, HC], F16)   # [hp, n, hc] pair-major h
            nc.sync.dma_start(hTn[:], hTn_d[:])
            w4 = cpool.tile([128, HC, H], F16)    # [hp, hc, k]
            nc.sync.dma_start(w4[:], W4_d[:])
            A16 = cpool.tile([128, H], F16)       # (h@W13 + bW)[n, k]
            nc.sync.dma_start(A16[:], A16_d[:])
            Bm16 = cpool.tile([128, H], F16)      # (h@W23)[n, k]
            nc.sync.dma_start(Bm16[:], Bm16_d[:])
            hhat = cpool.tile([128, HC], F16)
            nc.sync.dma_start(hhat[:], hhat_d[:])
            ident = cpool.tile([128, 128], F16)
            nc.sync.dma_start(ident[:], id_d[:])

            ones16 = cpool.tile([1, 128], F16)
            nc.vector.memset(ones16[:], 1.0)
            ones128 = cpool.tile([128, PT], F16)
            nc.vector.memset(ones128[:], 1.0)

            # ---- fused pass: spansT + z + online softmax + weighted sum ----
            # Flash-style: per tile keep running max m, denominator D, and
            # rescaled accumulator acc; no full spansT residency.
            segs_by_tile = [[] for _ in range(NT)]
            for (t, i, j0, p0, ln) in _SEGS:
                segs_by_tile[t].append((i, j0, p0, ln))

            acc = cpool.tile([128, HC], F32)
            nc.vector.memset(acc[:], 0.0)
            dsum = cpool.tile([1, 1], F32)
            nc.vector.memset(dsum[:], 0.0)
            m_tiles = [cpool.tile([1, 1], F32, name=f"m{k}") for k in range(2)]
            nc.vector.memset(m_tiles[0][:], -3.0e38)

            for t in range(nt):
                wt = _tile_width(t)
                # Build the selection tile on device: sa[n, p] = 1 iff n == i_p
                # (row runs), sb[n, p] = 1 iff n == j_p (shifted diagonals ==
                # column slices of the identity).
                selt = wpool.tile([128, 2, PT], F16, tag="sel", bufs=2)
                sa = selt[:, 0, :]
                sb = selt[:, 1, :]
                for (i, j0, p0, ln) in segs_by_tile[t]:
                    ia, ib = bass.broadcast_tensor_aps(ident[:, i:i + 1],
                                                       ones128[:, :ln])
                    nc.vector.tensor_tensor(sa[:, p0:p0 + ln], ia, ib, OP.mult)
                    nc.vector.tensor_copy(sb[:, p0:p0 + ln], ident[:, j0:j0 + ln])
                prod = wpool.tile([128, PT, HC], F16, tag="prod", bufs=2)
                # pair-major prod: one dense broadcast multiply per segment
                # covers all 6 h-chunks (contiguous [128, len*HC] out)
                for (i, j0, p0, ln) in segs_by_tile[t]:
                    in0 = hTn[:, j0:j0 + ln, :]
                    in1 = hTn[:, i:i + 1, :]
                    in0b, in1b = bass.broadcast_tensor_aps(in0, in1)
                    nc.vector.tensor_tensor(prod[:, p0:p0 + ln, :], in0b, in1b,
                                            OP.mult)
                spans = wpool.tile([128, HC, PT], F16, tag="spans", bufs=2)
                for kc in range(HC):
                    kcs = slice(kc * 128, (kc + 1) * 128)
                    ps = mpsum.tile([128, PT], F32, tag="sp", bufs=4)
                    for hc in range(HC):
                        nc.tensor.matmul(ps[:, :wt], w4[:, hc, kcs],
                                         prod[:, :wt, hc],
                                         start=(hc == 0), stop=False)
                    nc.tensor.matmul(ps[:, :wt], A16[:, kcs], sa[:, :wt],
                                     start=False, stop=False)
                    nc.tensor.matmul(ps[:, :wt], Bm16[:, kcs], sb[:, :wt],
                                     start=False, stop=True)
                    nc.scalar.activation(spans[:, kc, :wt], ps[:, :wt], AF.Tanh)
                zp = mpsum.tile([1, PT], F32, tag="zp", bufs=2)
                for kc in range(HC):
                    nc.tensor.matmul(zp[:1, :wt], hhat[:, kc:kc + 1],
                                     spans[:, kc, :wt],
                                     start=(kc == 0), stop=(kc == HC - 1))
                zrow = wpool.tile([1, PT], F32, tag="zrow", bufs=2)
                nc.vector.tensor_copy(zrow[:1, :wt], zp[:1, :wt])

                # online softmax update
                m_old = m_tiles[t % 2]
                m_new = m_tiles[(t + 1) % 2]
                mt = wpool.tile([1, 1], F32, tag="mt", bufs=2)
                nc.vector.tensor_reduce(mt[:], zrow[:1, :wt],
                                        mybir.AxisListType.X, OP.max)
                nc.vector.tensor_max(m_new[:], m_old[:], mt[:])
                negm = wpool.tile([1, 1], F32, tag="negm", bufs=2)
                nc.vector.tensor_scalar_mul(negm[:], m_new[:], -1.0)
                resc = wpool.tile([1, 1], F32, tag="resc", bufs=2)
                nc.scalar.activation(resc[:], m_old[:], AF.Exp, bias=negm[:])
                e16row = wpool.tile([1, PT], F16, tag="e16", bufs=2)
                dt_ = wpool.tile([1, 1], F32, tag="dt", bufs=2)
                nc.scalar.activation(e16row[:1, :wt], zrow[:1, :wt], AF.Exp,
                                     bias=negm[:], accum_out=dt_[:])
                nc.vector.tensor_scalar(dsum[:], dsum[:], resc[:], None, OP.mult)
                nc.vector.tensor_add(dsum[:], dsum[:], dt_[:])
                # broadcast resc across partitions via ones-outer matmul
                resc16 = wpool.tile([1, 1], F16, tag="resc16", bufs=2)
                nc.scalar.copy(resc16[:], resc[:])
                rbp = mpsum.tile([128, 1], F32, tag="rb", bufs=2)
                nc.tensor.matmul(rbp[:, :1], ones16[:1, :], resc16[:1, :1],
                                 start=True, stop=True)
                rb = wpool.tile([128, 1], F32, tag="rbs", bufs=2)
                nc.vector.tensor_copy(rb[:], rbp[:])
                nc.vector.tensor_scalar(acc[:], acc[:], rb[:], None, OP.mult)

                # weighted accumulation of this tile's spans
                ebp = mpsum.tile([128, PT], F32, tag="sp", bufs=4)
                nc.tensor.matmul(ebp[:, :wt], ones16[:1, :], e16row[:1, :wt],
                                 start=True, stop=True)
                eb16 = wpool.tile([128, PT], F16, tag="eb16", bufs=2)
                if t % 2 == 0:
                    nc.vector.tensor_copy(eb16[:, :wt], ebp[:, :wt])
                else:
                    nc.scalar.copy(eb16[:, :wt], ebp[:, :wt])
                for kc in range(HC):
                    tmp16 = wpool.tile([128, PT], F16, tag="tmp16", bufs=2)
                    nc.vector.tensor_mul(tmp16[:, :wt], spans[:, kc, :wt],
                                         eb16[:, :wt])
                    part = wpool.tile([128, 1], F32, tag="part", bufs=2)
                    if (t * HC + kc) % 2 == 0:
                        scrap = wpool.tile([128, PT], F16, tag="scrap", bufs=2)
                        nc.scalar.activation(scrap[:, :wt], tmp16[:, :wt],
                                             AF.Identity, accum_out=part[:])
                    else:
                        nc.vector.tensor_reduce(part[:], tmp16[:, :wt],
                                                mybir.AxisListType.X, OP.add)
                    nc.vector.tensor_add(acc[:, kc:kc + 1], acc[:, kc:kc + 1],
                                         part[:])

            nc.sync.dma_start(out_d[:, :HC], acc[:])
            nc.sync.dma_start(out_d[:1, HC:HC + 1], dsum[:])
    nc.finalize()
    return nc


_NC_CACHE = None


def _get_nc():
    global _NC_CACHE
    if _NC_CACHE is None:
        _NC_CACHE = build_nc()
    return _NC_CACHE


_IDENT = np.eye(128, dtype=NPMM)


def prepare_in_maps(h, Wcat, bW, h_hat, Wout, bout):
    h = np.asarray(h, np.float32)
    Wcat = np.asarray(Wcat, np.float32)
    bW = np.asarray(bW, np.float32)
    h_hat = np.asarray(h_hat, np.float32)

    W1, W2, W3, W4 = np.split(Wcat, 4, axis=0)
    W13 = W1 + W3
    W23 = W2 - W3
    # Host precompute of the per-token linear terms (saves shipping W13/W23):
    hf = h.reshape(B * N, H)
    A = (hf @ W13 + bW).reshape(B, N, H).astype(NPMM)
    Bm = (hf @ W23).reshape(B, N, H).astype(NPMM)

    W4p = np.ascontiguousarray(
        W4.astype(NPMM).reshape(HC, 128, H).transpose(1, 0, 2))  # [p, c, k]
    hhatT = np.ascontiguousarray(h_hat.astype(NPMM).reshape(HC, 128).T)

    in_maps = []
    for b in range(B):
        hb16 = h[b].astype(NPMM)
        in_maps.append({
            "hTn": np.ascontiguousarray(
                hb16.reshape(N, HC, 128).transpose(2, 0, 1)),  # [p, n, c]
            "W4p": W4p,
            "A16": np.ascontiguousarray(A[b]),
            "Bm16": np.ascontiguousarray(Bm[b]),
            "hhatT": hhatT,
            "ident": _IDENT,
        })
    return in_maps


def kernel(**inputs) -> np.ndarray:
    Wout = np.asarray(inputs["Wout"], np.float32)
    bout = np.asarray(inputs["bout"], np.float32)
    in_maps = prepare_in_maps(**inputs)
    nc = _get_nc()
    res = run_bass_kernel_spmd(nc, in_maps, list(range(B)))
    global _LAST_RES
    _LAST_RES = res

    out = np.zeros((B, C), np.float32)
    for b in range(B):
        o = res.results[b]["out"]                # [128, HC+1]
        acc = o[:, :HC]
        D = float(o[0, HC])
        ht = np.ascontiguousarray(acc.T).reshape(H) / D
        logits = ht @ Wout + bout
        m = logits.max()
        out[b] = logits - m - np.log(np.exp(logits - m).sum())
    return out



# revision 7
# speedup vs baseline: 1.8864x; 1.8864x over previous
"""Trainium2 Bass kernel for the pair-span GNN message-passing model, v2.

Math (per batch element b):
    W1..W4 = split(Wcat); A' = h @ (W1+W3) + bW;  Bm = h @ (W2-W3)
    For each triu pair p=(i,j):  spans[p] = tanh(A'[i] + Bm[j] + (h_i*h_j) @ W4)
    alpha = softmax(spans @ h_hat);  h_tilde = alpha^T spans
    out = log_softmax(h_tilde @ Wout + bout)

Sharding: data-parallel over batch B=8 across the 8 NeuronCores.

v2 minimizes host->device traffic (the dominant cost in this setup):
  - A' and Bm are precomputed on host (cheap BLAS) so W13/W23 never ship.
  - The 0/1 pair-selection matrices are built on device per tile (row
    memsets for the i-run selector, identity-column copies for the
    shifted-diagonal j selector) instead of shipping 4.5MB per core.
  - Single merged output tensor [128, HC+1] (acc columns + denominator).
  - No gpsimd: the softmax rescale broadcast uses a ones-outer matmul.
Per-core inputs: hTn 196KB + A16/Bm16 392KB + W4 1.18MB + ident 32KB.
"""

import ml_dtypes
import numpy as np

import concourse.bacc as bacc
import concourse.bass as bass
import concourse.mybir as mybir
from concourse import tile as tile_mod
from concourse.bass_utils import run_bass_kernel_spmd

B, N, H, C = 8, 128, 768, 5
HC = H // 128          # 6 chunks of the hidden dim
P_TOT = N * (N + 1) // 2   # 8256 pairs
PT = 512               # pairs per tile
NT = (P_TOT + PT - 1) // PT  # 17 tiles (last has 64 pairs)

F16 = mybir.dt.bfloat16    # matmul operand dtype: bf16 runs 1 cyc/row on PE
F32 = mybir.dt.float32     # (fp16 measured 2 cyc/row on HW)
NPMM = ml_dtypes.bfloat16  # host dtype for matmul operands


def _tile_width(t: int) -> int:
    return min(PT, P_TOT - t * PT)


def _segments():
    """(tile, i, j0, p0_local, length) runs of pairs; block i = pairs (i, j>=i)."""
    segs = []
    p = 0
    for i in range(N):
        j0 = i
        rem = N - i
        while rem > 0:
            t, p0 = divmod(p, PT)
            take = min(rem, PT - p0)
            segs.append((t, i, j0, p0, take))
            p += take
            j0 += take
            rem -= take
    return segs


_SEGS = _segments()


def build_nc(nt: int = NT) -> bass.Bass:
    # Bacc (not plain Bass): its finalize() runs generate_event_semaphores,
    # which splits multi-sem waits into standalone InstEventSemaphore ops —
    # the DVE ISA structs only fit one wait each.
    nc = bacc.Bacc(None)
    AF = mybir.ActivationFunctionType
    OP = mybir.AluOpType

    hTn_d = nc.declare_dram_parameter("hTn", [128, HC, N], F16, isOutput=False)
    W4_d = nc.declare_dram_parameter("W4p", [128, HC, H], F16, isOutput=False)
    A16_d = nc.declare_dram_parameter("A16", [128, H], F16, isOutput=False)
    Bm16_d = nc.declare_dram_parameter("Bm16", [128, H], F16, isOutput=False)
    hhat_d = nc.declare_dram_parameter("hhatT", [128, HC], F16, isOutput=False)
    id_d = nc.declare_dram_parameter("ident", [128, 128], F16, isOutput=False)
    out_d = nc.declare_dram_parameter("out", [128, HC + 1], F32, isOutput=True)

    with tile_mod.TileContext(nc) as tc:
        with (
            tc.tile_pool(name="const", bufs=1) as cpool,
            tc.tile_pool(name="work", bufs=2) as wpool,
            tc.tile_pool(name="mpsum", bufs=1, space="PSUM") as mpsum,
        ):
            # ---- static loads (all fully contiguous) ----
            hTn = cpool.tile([128, HC, N], F16)   # [hp, hc, n] chunk-major h
            nc.sync.dma_start(hTn[:], hTn_d[:])
            w4 = cpool.tile([128, HC, H], F16)    # [hp, hc, k]
            nc.sync.dma_start(w4[:], W4_d[:])
            A16 = cpool.tile([128, H], F16)       # (h@W13 + bW)[n, k]
            nc.sync.dma_start(A16[:], A16_d[:])
            Bm16 = cpool.tile([128, H], F16)      # (h@W23)[n, k]
            nc.sync.dma_start(Bm16[:], Bm16_d[:])
            hhat = cpool.tile([128, HC], F16)
            nc.sync.dma_start(hhat[:], hhat_d[:])
            ident = cpool.tile([128, 128], F16)
            nc.sync.dma_start(ident[:], id_d[:])

            ones16 = cpool.tile([1, 128], F16)
            nc.vector.memset(ones16[:], 1.0)
            ones128 = cpool.tile([128, PT], F16)
            nc.vector.memset(ones128[:], 1.0)

            # ---- fused pass: spansT + z + online softmax + weighted sum ----
            # Flash-style: per tile keep running max m, denominator D, and
            # rescaled accumulator acc; no full spansT residency.
            segs_by_tile = [[] for _ in range(NT)]
            for (t, i, j0, p0, ln) in _SEGS:
                segs_by_tile[t].append((i, j0, p0, ln))

            acc = cpool.tile([128, HC], F32)
            nc.vector.memset(acc[:], 0.0)
            dsum = cpool.tile([1, 1], F32)
            nc.vector.memset(dsum[:], 0.0)
            m_tiles = [cpool.tile([1, 1], F32, name=f"m{k}") for k in range(2)]
            nc.vector.memset(m_tiles[0][:], -3.0e38)

            for t in range(nt):
                wt = _tile_width(t)
                # Build the selection tile on device: sa[n, p] = 1 iff n == i_p
                # (row runs), sb[n, p] = 1 iff n == j_p (shifted diagonals ==
                # column slices of the identity).
                selt = wpool.tile([128, 2, PT], F16, tag="sel", bufs=2)
                sa = selt[:, 0, :]
                sb = selt[:, 1, :]
                for (i, j0, p0, ln) in segs_by_tile[t]:
                    ia, ib = bass.broadcast_tensor_aps(ident[:, i:i + 1],
                                                       ones128[:, :ln])
                    nc.vector.tensor_tensor(sa[:, p0:p0 + ln], ia, ib, OP.mult)
                    nc.vector.tensor_copy(sb[:, p0:p0 + ln], ident[:, j0:j0 + ln])
                prod = wpool.tile([128, HC, PT], F16, tag="prod", bufs=2)
                # chunk-major prod: one broadcast multiply per segment covers
                # all 6 h-chunks; keeps prod[:, hc, :] contiguous so the PE
                # moving-operand read runs at full rate.
                for (i, j0, p0, ln) in segs_by_tile[t]:
                    in0 = hTn[:, :, j0:j0 + ln]
                    in1 = hTn[:, :, i:i + 1]
                    in0b, in1b = bass.broadcast_tensor_aps(in0, in1)
                    nc.vector.tensor_tensor(prod[:, :, p0:p0 + ln], in0b, in1b,
                                            OP.mult)
                spans = wpool.tile([128, HC, PT], F16, tag="spans", bufs=2)
                for kc in range(HC):
                    kcs = slice(kc * 128, (kc + 1) * 128)
                    ps = mpsum.tile([128, PT], F32, tag="sp", bufs=4)
                    for hc in range(HC):
                        nc.tensor.matmul(ps[:, :wt], w4[:, hc, kcs],
                                         prod[:, hc, :wt],
                                         start=(hc == 0), stop=False)
                    nc.tensor.matmul(ps[:, :wt], A16[:, kcs], sa[:, :wt],
                                     start=False, stop=False)
                    nc.tensor.matmul(ps[:, :wt], Bm16[:, kcs], sb[:, :wt],
                                     start=False, stop=True)
                    nc.scalar.activation(spans[:, kc, :wt], ps[:, :wt], AF.Tanh)
                zp = mpsum.tile([1, PT], F32, tag="zp", bufs=2)
                for kc in range(HC):
                    nc.tensor.matmul(zp[:1, :wt], hhat[:, kc:kc + 1],
                                     spans[:, kc, :wt],
                                     start=(kc == 0), stop=(kc == HC - 1))
                zrow = wpool.tile([1, PT], F32, tag="zrow", bufs=2)
                nc.vector.tensor_copy(zrow[:1, :wt], zp[:1, :wt])

                # online softmax update
                m_old = m_tiles[t % 2]
                m_new = m_tiles[(t + 1) % 2]
                mt = wpool.tile([1, 1], F32, tag="mt", bufs=2)
                nc.vector.tensor_reduce(mt[:], zrow[:1, :wt],
                                        mybir.AxisListType.X, OP.max)
                nc.vector.tensor_max(m_new[:], m_old[:], mt[:])
                negm = wpool.tile([1, 1], F32, tag="negm", bufs=2)
                nc.vector.tensor_scalar_mul(negm[:], m_new[:], -1.0)
                resc = wpool.tile([1, 1], F32, tag="resc", bufs=2)
                nc.scalar.activation(resc[:], m_old[:], AF.Exp, bias=negm[:])
                e16row = wpool.tile([1, PT], F16, tag="e16", bufs=2)
                dt_ = wpool.tile([1, 1], F32, tag="dt", bufs=2)
                nc.scalar.activation(e16row[:1, :wt], zrow[:1, :wt], AF.Exp,
                                     bias=negm[:], accum_out=dt_[:])
                nc.vector.tensor_scalar(dsum[:], dsum[:], resc[:], None, OP.mult)
                nc.vector.tensor_add(dsum[:], dsum[:], dt_[:])
                # broadcast resc across partitions via ones-outer matmul
                resc16 = wpool.tile([1, 1], F16, tag="resc16", bufs=2)
                nc.scalar.copy(resc16[:], resc[:])
                rbp = mpsum.tile([128, 1], F32, tag="rb", bufs=2)
                nc.tensor.matmul(rbp[:, :1], ones16[:1, :], resc16[:1, :1],
                                 start=True, stop=True)
                rb = wpool.tile([128, 1], F32, tag="rbs", bufs=2)
                nc.vector.tensor_copy(rb[:], rbp[:])
                nc.vector.tensor_scalar(acc[:], acc[:], rb[:], None, OP.mult)

                # weighted accumulation of this tile's spans
                ebp = mpsum.tile([128, PT], F32, tag="sp", bufs=4)
                nc.tensor.matmul(ebp[:, :wt], ones16[:1, :], e16row[:1, :wt],
                                 start=True, stop=True)
                eb16 = wpool.tile([128, PT], F16, tag="eb16", bufs=2)
                if t % 2 == 0:
                    nc.vector.tensor_copy(eb16[:, :wt], ebp[:, :wt])
                else:
                    nc.scalar.copy(eb16[:, :wt], ebp[:, :wt])
                for kc in range(HC):
                    tmp16 = wpool.tile([128, PT], F16, tag="tmp16", bufs=2)
                    nc.vector.tensor_mul(tmp16[:, :wt], spans[:, kc, :wt],
                                         eb16[:, :wt])
                    part = wpool.tile([128, 1], F32, tag="part", bufs=2)
                    if (t * HC + kc) % 2 == 0:
                        scrap = wpool.tile([128, PT], F16, tag="scrap", bufs=2)
                        nc.scalar.activation(scrap[:, :wt], tmp16[:, :wt],
                                             AF.Identity, accum_out=part[:])
                    else:
                        nc.vector.tensor_reduce(part[:], tmp16[:, :wt],
                                                mybir.AxisListType.X, OP.add)
                    nc.vector.tensor_add(acc[:, kc:kc + 1], acc[:, kc:kc + 1],
                                         part[:])

            nc.sync.dma_start(out_d[:, :HC], acc[:])
            nc.sync.dma_start(out_d[:1, HC:HC + 1], dsum[:])
    nc.finalize()
    return nc


_NC_CACHE = None


def _get_nc():
    global _NC_CACHE
    if _NC_CACHE is None:
        _NC_CACHE = build_nc()
    return _NC_CACHE


_IDENT = np.eye(128, dtype=NPMM)


def prepare_in_maps(h, Wcat, bW, h_hat, Wout, bout):
    h = np.asarray(h, np.float32)
    Wcat = np.asarray(Wcat, np.float32)
    bW = np.asarray(bW, np.float32)
    h_hat = np.asarray(h_hat, np.float32)

    W1, W2, W3, W4 = np.split(Wcat, 4, axis=0)
    W13 = W1 + W3
    W23 = W2 - W3
    # Host precompute of the per-token linear terms (saves shipping W13/W23):
    hf = h.reshape(B * N, H)
    A = (hf @ W13 + bW).reshape(B, N, H).astype(NPMM)
    Bm = (hf @ W23).reshape(B, N, H).astype(NPMM)

    W4p = np.ascontiguousarray(
        W4.astype(NPMM).reshape(HC, 128, H).transpose(1, 0, 2))  # [p, c, k]
    hhatT = np.ascontiguousarray(h_hat.astype(NPMM).reshape(HC, 128).T)

    in_maps = []
    for b in range(B):
        hb16 = h[b].astype(NPMM)
        in_maps.append({
            "hTn": np.ascontiguousarray(
                hb16.reshape(N, HC, 128).transpose(2, 1, 0)),  # [p, c, n]
            "W4p": W4p,
            "A16": np.ascontiguousarray(A[b]),
            "Bm16": np.ascontiguousarray(Bm[b]),
            "hhatT": hhatT,
            "ident": _IDENT,
        })
    return in_maps


def kernel(**inputs) -> np.ndarray:
    Wout = np.asarray(inputs["Wout"], np.float32)
    bout = np.asarray(inputs["bout"], np.float32)
    in_maps = prepare_in_maps(**inputs)
    nc = _get_nc()
    res = run_bass_kernel_spmd(nc, in_maps, list(range(B)))
    global _LAST_RES
    _LAST_RES = res

    out = np.zeros((B, C), np.float32)
    for b in range(B):
        o = res.results[b]["out"]                # [128, HC+1]
        acc = o[:, :HC]
        D = float(o[0, HC])
        ht = np.ascontiguousarray(acc.T).reshape(H) / D
        logits = ht @ Wout + bout
        m = logits.max()
        out[b] = logits - m - np.log(np.exp(logits - m).sum())
    return out



# revision 8
# speedup vs baseline: 2.7280x; 1.4461x over previous
"""Trainium2 Bass kernel for the pair-span GNN message-passing model, v3.

Math (per batch element b):
    W1..W4 = split(Wcat); A' = h @ (W1+W3) + bW;  Bm = h @ (W2-W3)
    For each triu pair p=(i,j):  spans[p] = tanh(A'[i] + Bm[j] + (h_i*h_j) @ W4)
    alpha = softmax(spans @ h_hat);  h_tilde = alpha^T spans
    out = log_softmax(h_tilde @ Wout + bout)

Sharding: data-parallel over batch B=8 across the 8 NeuronCores.

v3 (vs v2, 604us -> target ~200us):
  - bf16 matmul operands (fp16 measured 2 cyc/row on HW; bf16 is 1).
  - prod (h_i*h_j) and AB (A'[i]+Bm[j]) are precomputed on host and
    streamed per tile from HBM in chunk-major layout, so the DVE no
    longer builds them (DVE was a 205us co-bottleneck) and the PE
    moving operands are contiguous.
  - The A/B selection matmuls (12/tile) collapse to one identity
    matmul per output chunk (6/tile) that adds AB into PSUM.
  - Per-tile LOCAL softmax (own max m_t, unnormalized sums) with a
    single final combine: h_tilde = sum_t e^{m_t-M} part_t / sum_t
    e^{m_t-M} d_t.  No cross-tile serial chain, so the PE never waits
    on the softmax scalar ops (v2 lost ~4us/tile to that chain).
  - The exp-weight broadcast matmul for tile t is emitted after tile
    t+1's span matmuls (1-tile software pipeline) so the scalar chain
    latency hides under the next tile's PE work.
Per-core host->HBM traffic: prodT+ABt ~26.7MB (streamed), W4 1.2MB.
"""

import ml_dtypes
import numpy as np

import concourse.bacc as bacc
import concourse.bass as bass
import concourse.mybir as mybir
from concourse import tile as tile_mod
from concourse.bass_utils import run_bass_kernel_spmd

B, N, H, C = 8, 128, 768, 5
HC = H // 128          # 6 chunks of the hidden dim
P_TOT = N * (N + 1) // 2   # 8256 pairs
PT = 512               # pairs per tile
NT = (P_TOT + PT - 1) // PT  # 17 tiles (last has 64 pairs)

F16 = mybir.dt.bfloat16
F32 = mybir.dt.float32
NPMM = ml_dtypes.bfloat16


def _tile_width(t: int) -> int:
    return min(PT, P_TOT - t * PT)


def build_nc(nt: int = NT) -> bass.Bass:
    # Bacc (not plain Bass): its finalize() runs generate_event_semaphores,
    # which splits multi-sem waits into standalone InstEventSemaphore ops —
    # the DVE ISA structs only fit one wait each.
    nc = bacc.Bacc(None)
    AF = mybir.ActivationFunctionType
    OP = mybir.AluOpType

    prod_d = nc.declare_dram_parameter("prodT", [NT, 128, HC, PT], F16,
                                       isOutput=False)
    ab_d = nc.declare_dram_parameter("ABt", [NT, 128, HC, PT], F16,
                                     isOutput=False)
    W4_d = nc.declare_dram_parameter("W4p", [128, HC, H], F16, isOutput=False)
    hhat_d = nc.declare_dram_parameter("hhatT", [128, HC], F16, isOutput=False)
    id_d = nc.declare_dram_parameter("ident", [128, 128], F16, isOutput=False)
    out_d = nc.declare_dram_parameter("out", [128, HC + 1], F32, isOutput=True)

    with tile_mod.TileContext(nc) as tc:
        with (
            tc.tile_pool(name="const", bufs=1) as cpool,
            tc.tile_pool(name="work", bufs=2) as wpool,
            tc.tile_pool(name="mpsum", bufs=1, space="PSUM") as mpsum,
        ):
            # ---- static loads (w4 split per-chunk for DMA parallelism) ----
            w4 = cpool.tile([128, HC, H], F16)
            for hc in range(HC):
                nc.sync.dma_start(w4[:, hc, :], W4_d[:, hc, :])
            hhat = cpool.tile([128, HC], F16)
            nc.sync.dma_start(hhat[:], hhat_d[:])
            ident = cpool.tile([128, 128], F16)
            nc.sync.dma_start(ident[:], id_d[:])
            ones16 = cpool.tile([1, 128], F16)
            nc.vector.memset(ones16[:], 1.0)

            # per-tile softmax state (written column t by tile t)
            mts = cpool.tile([1, NT], F32)
            dall = cpool.tile([1, NT], F32)
            part_all = cpool.tile([128, HC, NT], F32)

            def emit_back(t, wt, spans, e16):
                # exp-weight broadcast + weighted accumulation for tile t;
                # called after tile t+1's span matmuls are emitted so the
                # softmax chain hides under them.
                ebp = mpsum.tile([128, PT], F32, tag="eb", bufs=2)
                nc.tensor.matmul(ebp[:, :wt], ones16[:1, :], e16[:1, :wt],
                                 start=True, stop=True)
                eb16 = wpool.tile([128, PT], F16, tag="eb16", bufs=2)
                nc.scalar.copy(eb16[:, :wt], ebp[:, :wt])
                for kc in range(HC):
                    tmp16 = wpool.tile([128, PT], F16, tag="tmp16", bufs=2)
                    nc.vector.tensor_mul(tmp16[:, :wt], spans[:, kc, :wt],
                                         eb16[:, :wt])
                    scrap = wpool.tile([128, PT], F16, tag="scrap", bufs=2)
                    nc.scalar.activation(scrap[:, :wt], tmp16[:, :wt],
                                         AF.Identity,
                                         accum_out=part_all[:, kc, t:t + 1])

            pend = None
            for t in range(nt):
                wt = _tile_width(t)
                pr = wpool.tile([128, HC, PT], F16, tag="pr", bufs=3)
                nc.sync.dma_start(pr[:], prod_d[t])
                ab = wpool.tile([128, HC, PT], F16, tag="ab", bufs=3)
                nc.sync.dma_start(ab[:], ab_d[t])
                spans = wpool.tile([128, HC, PT], F16, tag="spans", bufs=3)
                for kc in range(HC):
                    kcs = slice(kc * 128, (kc + 1) * 128)
                    ps = mpsum.tile([128, PT], F32, tag="sp", bufs=4)
                    for hc in range(HC):
                        nc.tensor.matmul(ps[:, :wt], w4[:, hc, kcs],
                                         pr[:, hc, :wt],
                                         start=(hc == 0), stop=False)
                    nc.tensor.matmul(ps[:, :wt], ident[:], ab[:, kc, :wt],
                                     start=False, stop=True)
                    nc.scalar.activation(spans[:, kc, :wt], ps[:, :wt],
                                         AF.Tanh)
                zp = mpsum.tile([1, PT], F32, tag="zp", bufs=2)
                for kc in range(HC):
                    nc.tensor.matmul(zp[:1, :wt], hhat[:, kc:kc + 1],
                                     spans[:, kc, :wt],
                                     start=(kc == 0), stop=(kc == HC - 1))
                # local softmax scalars (off the PE critical path)
                nc.vector.tensor_reduce(mts[:1, t:t + 1], zp[:1, :wt],
                                        mybir.AxisListType.X, OP.max)
                negm = wpool.tile([1, 1], F32, tag="negm", bufs=2)
                nc.vector.tensor_scalar_mul(negm[:], mts[:1, t:t + 1], -1.0)
                e16 = wpool.tile([1, PT], F16, tag="e16", bufs=2)
                nc.scalar.activation(e16[:1, :wt], zp[:1, :wt], AF.Exp,
                                     bias=negm[:],
                                     accum_out=dall[:1, t:t + 1])
                if pend is not None:
                    emit_back(*pend)
                pend = (t, wt, spans, e16)
            emit_back(*pend)

            # ---- final combine: M = max m_t, f_t = exp(m_t - M) ----
            Mx = cpool.tile([1, 1], F32)
            nc.vector.tensor_reduce(Mx[:], mts[:1, :nt],
                                    mybir.AxisListType.X, OP.max)
            negM = cpool.tile([1, 1], F32)
            nc.vector.tensor_scalar_mul(negM[:], Mx[:], -1.0)
            f = cpool.tile([1, NT], F32)
            nc.scalar.activation(f[:1, :nt], mts[:1, :nt], AF.Exp,
                                 bias=negM[:])
            fd = cpool.tile([1, NT], F32)
            nc.vector.tensor_mul(fd[:1, :nt], f[:1, :nt], dall[:1, :nt])
            Dv = cpool.tile([1, 1], F32)
            nc.vector.tensor_reduce(Dv[:], fd[:1, :nt],
                                    mybir.AxisListType.X, OP.add)
            f16 = cpool.tile([1, NT], F16)
            nc.scalar.copy(f16[:1, :nt], f[:1, :nt])
            fbp = mpsum.tile([128, 1, NT], F32, tag="eb", bufs=2)
            nc.tensor.matmul(fbp[:, 0, :nt], ones16[:1, :], f16[:1, :nt],
                             start=True, stop=True)
            partf = cpool.tile([128, HC, NT], F32)
            b0, b1 = bass.broadcast_tensor_aps(part_all[:, :, :nt],
                                               fbp[:, :, :nt])
            nc.vector.tensor_tensor(partf[:, :, :nt], b0, b1, OP.mult)
            acc = cpool.tile([128, HC], F32)
            nc.vector.tensor_reduce(acc[:], partf[:, :, :nt],
                                    mybir.AxisListType.X, OP.add)
            nc.sync.dma_start(out_d[:, :HC], acc[:])
            nc.sync.dma_start(out_d[:1, HC:HC + 1], Dv[:])
    nc.finalize()
    return nc


_NC_CACHE = None


def _get_nc():
    global _NC_CACHE
    if _NC_CACHE is None:
        _NC_CACHE = build_nc()
    return _NC_CACHE


_IDENT = np.eye(128, dtype=NPMM)


def _pack_tiles(x32):
    """[P, H] f32 -> [NT, 128, HC, PT] bf16 (zero-padded, chunk-major)."""
    pad = np.zeros((NT * PT, H), np.float32)
    pad[:P_TOT] = x32
    return np.ascontiguousarray(
        pad.reshape(NT, PT, HC, 128).transpose(0, 3, 2, 1).astype(NPMM))


def prepare_in_maps(h, Wcat, bW, h_hat, Wout, bout):
    h = np.asarray(h, np.float32)
    Wcat = np.asarray(Wcat, np.float32)
    bW = np.asarray(bW, np.float32)
    h_hat = np.asarray(h_hat, np.float32)

    W1, W2, W3, W4 = np.split(Wcat, 4, axis=0)
    W13 = W1 + W3
    W23 = W2 - W3
    hf = h.reshape(B * N, H)
    A = (hf @ W13 + bW).reshape(B, N, H)
    Bm = (hf @ W23).reshape(B, N, H)

    ii, jj = np.triu_indices(N)

    W4p = np.ascontiguousarray(
        W4.astype(NPMM).reshape(HC, 128, H).transpose(1, 0, 2))  # [p, c, k]
    hhatT = np.ascontiguousarray(h_hat.astype(NPMM).reshape(HC, 128).T)

    in_maps = []
    for b in range(B):
        prod = h[b][ii] * h[b][jj]          # [P, H]
        AB = A[b][ii] + Bm[b][jj]           # [P, H]
        in_maps.append({
            "prodT": _pack_tiles(prod),
            "ABt": _pack_tiles(AB),
            "W4p": W4p,
            "hhatT": hhatT,
            "ident": _IDENT,
        })
    return in_maps


def kernel(**inputs) -> np.ndarray:
    Wout = np.asarray(inputs["Wout"], np.float32)
    bout = np.asarray(inputs["bout"], np.float32)
    in_maps = prepare_in_maps(**inputs)
    nc = _get_nc()
    res = run_bass_kernel_spmd(nc, in_maps, list(range(B)))
    global _LAST_RES
    _LAST_RES = res

    out = np.zeros((B, C), np.float32)
    for b in range(B):
        o = res.results[b]["out"]                # [128, HC+1]
        acc = o[:, :HC]
        D = float(o[0, HC])
        ht = np.ascontiguousarray(acc.T).reshape(H) / D
        logits = ht @ Wout + bout
        m = logits.max()
        out[b] = logits - m - np.log(np.exp(logits - m).sum())
    return out


# revision 9
# speedup vs baseline: 2.8900x; 1.0594x over previous
"""Trainium2 Bass kernel for the pair-span GNN message-passing model, v4.

Math (per batch element b):
    W1..W4 = split(Wcat); A' = h @ (W1+W3) + bW;  Bm = h @ (W2-W3)
    For each triu pair p=(i,j):  spans[p] = tanh(A'[i] + Bm[j] + (h_i*h_j) @ W4)
    alpha = softmax(spans @ h_hat);  h_tilde = alpha^T spans
    out = log_softmax(h_tilde @ Wout + bout)

Sharding: data-parallel over batch B=8 across the 8 NeuronCores.

v4 (vs v3 @ 221us):
  - ACT was 92%-occupied (co-bottleneck): tanh now processes 2 output
    chunks per op (paired PSUM banks), 4 of 6 weighted-sum reductions
    moved to DVE (separate part tiles per engine to avoid cross-engine
    write hazards), exp-weight broadcast moved from PE-matmul+copy to
    gpsimd.partition_broadcast (gpsimd was idle).
  - Final softmax combine across tiles moved to host (was an ~8us
    serial tail); kernel ships per-tile partial sums + maxes + denoms.
  - Startup: tile-0 prod/AB and W4 loads are chunked across DMA queues
    (W4 relaid [pp, kc, hc, 128] so each kc block is one contiguous
    DMA); first matmul waited 18.5us in v3.
"""

import ml_dtypes
import numpy as np

import concourse.bacc as bacc
import concourse.bass as bass
import concourse.bass_isa as bass_isa
import concourse.mybir as mybir
from concourse import tile as tile_mod
from concourse.bass_utils import run_bass_kernel_spmd

B, N, H, C = 8, 128, 768, 5
HC = H // 128          # 6 chunks of the hidden dim
P_TOT = N * (N + 1) // 2   # 8256 pairs
PT = 512               # pairs per tile
NT = (P_TOT + PT - 1) // PT  # 17 tiles (last has 64 pairs)
NKA = 2                # weighted-sum chunks accumulated on ACT
NKV = HC - NKA         # ... and on DVE

F16 = mybir.dt.bfloat16
F32 = mybir.dt.float32
NPMM = ml_dtypes.bfloat16

# out columns: part_act [2*NT] | part_dve [4*NT] | mts [NT] | dall [NT]
_OC_PA = NKA * NT
_OC_PD = _OC_PA + NKV * NT
_OC_M = _OC_PD + NT
_OC_D = _OC_M + NT


def _tile_width(t: int) -> int:
    return min(PT, P_TOT - t * PT)


def build_nc(nt: int = NT) -> bass.Bass:
    nc = bacc.Bacc(None)
    AF = mybir.ActivationFunctionType
    OP = mybir.AluOpType

    prod_d = nc.declare_dram_parameter("prodT", [NT, 128, HC, PT], F16,
                                       isOutput=False)
    ab_d = nc.declare_dram_parameter("ABt", [NT, 128, HC, PT], F16,
                                     isOutput=False)
    W4_d = nc.declare_dram_parameter("W4p", [128, HC, HC, 128], F16,
                                     isOutput=False)
    hhat_d = nc.declare_dram_parameter("hhatT", [128, HC], F16, isOutput=False)
    id_d = nc.declare_dram_parameter("ident", [128, 128], F16, isOutput=False)
    out_d = nc.declare_dram_parameter("out", [128, _OC_D], F32, isOutput=True)

    with tile_mod.TileContext(nc) as tc:
        with (
            tc.tile_pool(name="const", bufs=1) as cpool,
            tc.tile_pool(name="work", bufs=2) as wpool,
            tc.tile_pool(name="mpsum", bufs=1, space="PSUM") as mpsum,
        ):
            # ---- tile-0 inputs + weights, chunked across DMA queues ----
            pr0 = wpool.tile([128, HC, PT], F16, tag="pr", bufs=4)
            for hc in range(HC):
                nc.sync.dma_start(pr0[:, hc, :], prod_d[0, :, hc, :])
            w4 = cpool.tile([128, HC, HC, 128], F16)  # [pp, kc, hc, c]
            for kc in range(HC):
                nc.sync.dma_start(w4[:, kc], W4_d[:, kc])
            ab0 = wpool.tile([128, HC, PT], F16, tag="ab", bufs=4)
            for hc in range(HC):
                nc.sync.dma_start(ab0[:, hc, :], ab_d[0, :, hc, :])
            hhat = cpool.tile([128, HC], F16)
            nc.sync.dma_start(hhat[:], hhat_d[:])
            ident = cpool.tile([128, 128], F16)
            nc.sync.dma_start(ident[:], id_d[:])

            # per-tile softmax state (tile t writes column t)
            mts = cpool.tile([1, NT], F32)
            dall = cpool.tile([1, NT], F32)
            part_act = cpool.tile([128, NKA, NT], F32)
            part_dve = cpool.tile([128, NKV, NT], F32)

            def front(t, pr, ab):
                wt = _tile_width(t)
                spans = wpool.tile([128, HC, PT], F16, tag="spans", bufs=4)
                for kcp in range(HC // 2):
                    ps2 = mpsum.tile([128, 2, PT], F32, tag="sp", bufs=2)
                    for sub in range(2):
                        kc = 2 * kcp + sub
                        for hc in range(HC):
                            nc.tensor.matmul(ps2[:, sub, :wt],
                                             w4[:, kc, hc, :],
                                             pr[:, hc, :wt],
                                             start=(hc == 0), stop=False)
                        nc.tensor.matmul(ps2[:, sub, :wt], ident[:],
                                         ab[:, kc, :wt],
                                         start=False, stop=True)
                    nc.scalar.activation(
                        spans[:, 2 * kcp:2 * kcp + 2, :wt],
                        ps2[:, :, :wt], AF.Tanh)
                zp = mpsum.tile([1, PT], F32, tag="zp", bufs=2)
                for kc in range(HC):
                    nc.tensor.matmul(zp[:1, :wt], hhat[:, kc:kc + 1],
                                     spans[:, kc, :wt],
                                     start=(kc == 0), stop=(kc == HC - 1))
                nc.vector.tensor_reduce(mts[:1, t:t + 1], zp[:1, :wt],
                                        mybir.AxisListType.X, OP.max)
                negm = wpool.tile([1, 1], F32, tag="negm", bufs=2)
                nc.vector.tensor_scalar_mul(negm[:], mts[:1, t:t + 1], -1.0)
                e16 = wpool.tile([1, PT], F16, tag="e16", bufs=3)
                nc.scalar.activation(e16[:1, :wt], zp[:1, :wt], AF.Exp,
                                     bias=negm[:],
                                     accum_out=dall[:1, t:t + 1])
                return t, wt, spans, e16

            def back(t, wt, spans, e16):
                eb16 = wpool.tile([128, PT], F16, tag="eb16", bufs=2)
                nc.gpsimd.partition_broadcast(eb16[:, :wt], e16[:1, :wt])
                for kc in range(HC):
                    tmp16 = wpool.tile([128, PT], F16, tag="tmp16", bufs=2)
                    nc.vector.tensor_mul(tmp16[:, :wt], spans[:, kc, :wt],
                                         eb16[:, :wt])
                    if kc < NKA:
                        scrap = wpool.tile([128, PT], F16, tag="scrap",
                                           bufs=2)
                        nc.scalar.activation(
                            scrap[:, :wt], tmp16[:, :wt], AF.Identity,
                            accum_out=part_act[:, kc, t:t + 1])
                    else:
                        nc.vector.tensor_reduce(
                            part_dve[:, kc - NKA, t:t + 1], tmp16[:, :wt],
                            mybir.AxisListType.X, OP.add)

            pend = None
            for t in range(nt):
                if t == 0:
                    pr, ab = pr0, ab0
                else:
                    pr = wpool.tile([128, HC, PT], F16, tag="pr", bufs=4)
                    nc.sync.dma_start(pr[:], prod_d[t])
                    ab = wpool.tile([128, HC, PT], F16, tag="ab", bufs=4)
                    nc.sync.dma_start(ab[:], ab_d[t])
                cur = front(t, pr, ab)
                if pend is not None:
                    back(*pend)
                pend = cur
            back(*pend)

            nc.sync.dma_start(out_d[:, :_OC_PA], part_act[:])
            nc.sync.dma_start(out_d[:, _OC_PA:_OC_PD], part_dve[:])
            nc.sync.dma_start(out_d[:1, _OC_PD:_OC_M], mts[:1, :])
            nc.sync.dma_start(out_d[:1, _OC_M:_OC_D], dall[:1, :])
    nc.finalize()
    return nc


_NC_CACHE = None


def _get_nc():
    global _NC_CACHE
    if _NC_CACHE is None:
        _NC_CACHE = build_nc()
    return _NC_CACHE


_IDENT = np.eye(128, dtype=NPMM)


def _pack_tiles(x32):
    """[P, H] f32 -> [NT, 128, HC, PT] bf16 (zero-padded, chunk-major)."""
    pad = np.zeros((NT * PT, H), np.float32)
    pad[:P_TOT] = x32
    return np.ascontiguousarray(
        pad.reshape(NT, PT, HC, 128).transpose(0, 3, 2, 1).astype(NPMM))


def prepare_in_maps(h, Wcat, bW, h_hat, Wout, bout):
    h = np.asarray(h, np.float32)
    Wcat = np.asarray(Wcat, np.float32)
    bW = np.asarray(bW, np.float32)
    h_hat = np.asarray(h_hat, np.float32)

    W1, W2, W3, W4 = np.split(Wcat, 4, axis=0)
    hf = h.reshape(B * N, H)
    A = (hf @ (W1 + W3) + bW).reshape(B, N, H)
    Bm = (hf @ (W2 - W3)).reshape(B, N, H)

    ii, jj = np.triu_indices(N)

    # W4p[pp, kc, hc, c] = W4[hc*128+pp, kc*128+c]
    W4p = np.ascontiguousarray(
        W4.astype(NPMM).reshape(HC, 128, HC, 128).transpose(1, 2, 0, 3))
    hhatT = np.ascontiguousarray(h_hat.astype(NPMM).reshape(HC, 128).T)

    in_maps = []
    for b in range(B):
        prod = h[b][ii] * h[b][jj]          # [P, H]
        AB = A[b][ii] + Bm[b][jj]           # [P, H]
        in_maps.append({
            "prodT": _pack_tiles(prod),
            "ABt": _pack_tiles(AB),
            "W4p": W4p,
            "hhatT": hhatT,
            "ident": _IDENT,
        })
    return in_maps


def kernel(**inputs) -> np.ndarray:
    Wout = np.asarray(inputs["Wout"], np.float32)
    bout = np.asarray(inputs["bout"], np.float32)
    in_maps = prepare_in_maps(**inputs)
    nc = _get_nc()
    res = run_bass_kernel_spmd(nc, in_maps, list(range(B)))
    global _LAST_RES
    _LAST_RES = res

    out = np.zeros((B, C), np.float32)
    for b in range(B):
        o = res.results[b]["out"]                # [128, _OC_D]
        pa = o[:, :_OC_PA].reshape(128, NKA, NT)
        pd = o[:, _OC_PA:_OC_PD].reshape(128, NKV, NT)
        part = np.concatenate([pa, pd], axis=1)  # [128, HC, NT]
        mts = o[0, _OC_PD:_OC_M]
        dall = o[0, _OC_M:_OC_D]
        f = np.exp(mts - mts.max())
        D = float((f * dall).sum())
        acc = (part * f).sum(-1)                 # [128, HC]
        ht = np.ascontiguousarray(acc.T).reshape(H) / D
        logits = ht @ Wout + bout
        m = logits.max()
        out[b] = logits - m - np.log(np.exp(logits - m).sum())
    return out
